# revision 29
# baseline (speedup 1.0000x reference)
"""Trainium2 Bass kernel for an EquivariantProductBasisBlock (MACE-style
symmetric contraction + per-irrep linear + residual).

Problem shapes (hardcoded):
  N=1024 atoms, C=64 channels, K=9 total irrep dim, irreps dims (1,3,5),
  paths P3=4/P2=2/P1=1, E=10 species, 8 NeuronCores, data-parallel over N.

Decomposition (validated exactly against the reference):
  per sample s=(n,c), x = node_feats[n,c,:] (9-vec), per-irrep r species
  weights w3(4)/w2(2)/w1(1) gathered host-side into Wg[n,c,21]:
    T3[d] = sum_{a,b,i,p} U3[d,a,b,i,p] x_a x_b x_i w3_p
    T2[d] = sum_{a,i,p}   U2[d,a,i,p]   x_a x_i w2_p
    T1[d] = sum_{a}       U1[d,a,0]     x_a w1
  On device:  V[s] = [y2 = x (x) x (81) | pad | x (9)]  (sample-major fp16,
  PE-transposed per 128-sample chunk), then
    H[s, 0:360] = V @ Uall   (fp16 matmul, fp32 PSUM, ACT-copied to SBUF)
    step2 on DVE (fp16 2x mode): products [36 T3 | 4 T2+T1] per output d,
    three pairwise-add levels + one 5-wide reduce -> o_all
  Then per-irrep linear via 4 fp16 matmuls (contraction over channels) with
  fused +sc adds from PSUM; restage DMAs overlap phase 2 per atom-half.
"""

import numpy as np

N_ATOMS = 1024
C = 64
K = 9
DIMS = (1, 3, 5)
DOFF = (0, 1, 4)
P3, P2, P1 = 4, 2, 1
N_CORES = 8
APC = N_ATOMS // N_CORES  # atoms per core = 128
NB = APC // 2  # chunks per core = 64 (each chunk = 2 atoms x 64 channels)
GROUP = 4  # chunks per PSUM group
NGRP = NB // GROUP
SG = 8  # chunks per step2 supergroup
NSG = NB // SG

_compiled = {}


def _build_program():
    import concourse.bass as bass  # noqa: F401
    import concourse.tile as tile
    from concourse import bacc
    from concourse import mybir
    from concourse.masks import make_identity

    f32 = mybir.dt.float32
    f16 = mybir.dt.float16
    AX = mybir.AxisListType.X

    nc = bacc.Bacc(None)
    xarr_d = nc.dram_tensor("xarr", [128, NB, 9], f32, kind="ExternalInput")
    wg_d = nc.dram_tensor("wgarr", [128, NB, 22], f32, kind="ExternalInput")
    sc_d = nc.dram_tensor("scarr", [64, APC, 9], f32, kind="ExternalInput")
    uall_d = nc.dram_tensor("uall", [91, 360], f16, kind="ExternalInput")
    wlin_d = nc.dram_tensor("wlin", [64, 192], f16, kind="ExternalInput")
    yout_d = nc.dram_tensor("yout", [64, APC, 9], f32, kind="ExternalOutput")

    with tile.TileContext(nc) as tc:
        with (
            tc.tile_pool(name="const", bufs=1) as const,
            tc.tile_pool(name="big", bufs=1) as big,
            tc.tile_pool(name="vt", bufs=8) as vtp,
            tc.tile_pool(name="st2", bufs=2) as st2,
        ):
            ident16 = const.tile([128, 128], f16)
            make_identity(nc, ident16)
            ident32 = const.tile([128, 128], f32)
            make_identity(nc, ident32)
            u_sb = const.tile([91, 360], f16)
            nc.sync.dma_start(u_sb[:], uall_d[:])
            wlin_sb = const.tile([64, 192], f16)
            nc.sync.dma_start(wlin_sb[:], wlin_d[:])
            x_sb = const.tile([128, NB, 9], f32)
            nc.sync.dma_start(x_sb[:], xarr_d[:])
            wg_sb = const.tile([128, NB, 22], f32)
            nc.sync.dma_start(wg_sb[:], wg_d[:])
            sc_sb = const.tile([64, APC, 9], f32)
            nc.sync.dma_start(sc_sb[:], sc_d[:])

            vbig = big.tile([128, NB, 92], f16)
            ostage = big.tile([64, APC, 9], f16)
            ost_v = ostage.rearrange("p (n j) d -> p n j d", j=2)
            rallx = big.tile([128, NB, 108], f16)
            o_all = big.tile([128, NB, 9], f16)
            hsb = big.tile([128, NB, 360], f16)

            # ---- Phase 1: feature build (sample-major) ----
            # x rows of V at 4B-aligned offset 82; col 81 zeroed (junk
            # would hit the zero row of Uall otherwise; 0*NaN = NaN)
            nc.gpsimd.memset(vbig[:, :, 81:82], 0.0)
            nc.vector.tensor_copy(vbig[:, :, 82:91], x_sb)
            # y2[(a,b)] = x_a * x_b  (fp16 out for the PE); split so the
            # PE transposes can start after the first half
            for h in range(4):
                hk = NB // 4
                nc.vector.tensor_mul(
                    vbig[:, h * hk : (h + 1) * hk, 0:81].rearrange(
                        "p k (a b) -> p k a b", b=9
                    ),
                    x_sb[:, h * hk : (h + 1) * hk, :, None].broadcast_to(
                        (128, hk, 9, 9)
                    ),
                    x_sb[:, h * hk : (h + 1) * hk, None, :].broadcast_to(
                        (128, hk, 9, 9)
                    ),
                )

            # ---- Phase 2: pipelined PE transpose + matmul; h to SBUF ----
            with (
                tc.tile_pool(name="hps", bufs=3, space="PSUM") as hps,
                tc.tile_pool(name="vtps", bufs=2, space="PSUM") as vtps,
            ):
                # Warm-up PE ops absorb one-time GPSIMD (ident) / DMA (u_sb)
                # deps and start HAM warmup.
                warm = vtps.tile([92, 4, 128], f16, tag="vt_ps")
                nc.tensor.transpose(warm[:, 0, :], ident16[:, 0:92], ident16)
                warm2 = vtps.tile([128, 91], f16, tag="vt_ps")
                nc.tensor.transpose(warm2, u_sb[:, 0:128], ident16[0:91, 0:91])

                def emit_transposes(g):
                    vt_ps = vtps.tile([92, 4, 128], f16, tag="vt_ps")
                    for j in range(4):
                        nc.tensor.transpose(
                            vt_ps[:, j, :], vbig[:, g * 4 + j, :], ident16
                        )
                    vt_sb = vtp.tile([92, 4, 128], f16)
                    nc.scalar.copy(vt_sb, vt_ps)
                    return vt_sb

                def emit_step2(sg):
                    # t3t[dg] holds [36 T3 | 2 T2 | 1 T1] products; pairwise
                    # pre-add then one 20-wide DVE reduce per irrep writes
                    # o_all directly.
                    ks = sg * SG
                    t3t = st2.tile([128, SG, 9, 40], f16)
                    # rallx[(r,i,p)] = x_i * w3^r_p for this supergroup
                    for r in range(3):
                        nc.vector.tensor_mul(
                            rallx[
                                :, ks : ks + SG, r * 36 : (r + 1) * 36
                            ].rearrange("p k (i q) -> p k i q", q=4),
                            x_sb[:, ks : ks + SG, :, None].broadcast_to(
                                (128, SG, 9, 4)
                            ),
                            wg_sb[:, ks : ks + SG, None, r * 7 : r * 7 + 4]
                            .broadcast_to((128, SG, 9, 4)),
                        )
                    for r in range(3):
                        D, off = DIMS[r], DOFF[r]
                        h3 = hsb[
                            :, ks : ks + SG, off * 36 : (off + D) * 36
                        ].rearrange("p k (d f) -> p k d f", f=36)
                        m3 = rallx[:, ks : ks + SG, r * 36 : (r + 1) * 36][
                            :, :, None, :
                        ].broadcast_to((128, SG, D, 36))
                        nc.vector.tensor_mul(
                            t3t[:, :, off : off + D, 0:36], h3, m3
                        )
                        h21 = hsb[
                            :, ks : ks + SG, 324 + off * 4 : 324 + (off + D) * 4
                        ].rearrange("p k (d f) -> p k d f", f=4)
                        m21 = wg_sb[:, ks : ks + SG, r * 7 + 4 : r * 7 + 8][
                            :, :, None, :
                        ].broadcast_to((128, SG, D, 4))
                        nc.vector.tensor_mul(
                            t3t[:, :, off : off + D, 36:40], h21, m21
                        )
                    nc.vector.tensor_add(
                        t3t[:, :, :, 0:20],
                        t3t[:, :, :, 0:20],
                        t3t[:, :, :, 20:40],
                    )
                    nc.vector.tensor_add(
                        t3t[:, :, :, 0:10],
                        t3t[:, :, :, 0:10],
                        t3t[:, :, :, 10:20],
                    )
                    nc.vector.tensor_add(
                        t3t[:, :, :, 0:5],
                        t3t[:, :, :, 0:5],
                        t3t[:, :, :, 5:10],
                    )
                    with nc.allow_low_precision(
                        reason="5-term fp16 segment sum, O(1) values"
                    ):
                        nc.vector.reduce_sum(
                            out=o_all[:, ks : ks + SG, :],
                            in_=t3t[:, :, :, 0:5],
                            axis=AX,
                        )

                vt_cur = emit_transposes(0)
                for g in range(NGRP):
                    vt_nxt = emit_transposes(g + 1) if g + 1 < NGRP else None
                    for half in range(2):
                        ks = g * 4 + half * 2
                        h_t = hps.tile([128, 2, 512], f32)
                        for j in range(2):
                            nc.tensor.matmul(
                                h_t[:, j, 0:360],
                                vt_cur[0:91, half * 2 + j, :],
                                u_sb,
                                start=True,
                                stop=True,
                            )
                        nc.scalar.copy(
                            hsb[:, ks : ks + 2, 0:360], h_t[:, :, 0:360]
                        )
                    vt_cur = vt_nxt
                    if g % 2 == 1:
                        emit_step2(g // 2)
                        if g // 2 in (3, 7):
                            hh = (g // 2) // 4  # atom half 0 or 1
                            nc.sync.dma_start(
                                ost_v[:, hh * 32 : hh * 32 + 32, 0, :],
                                o_all[0:64, hh * 32 : hh * 32 + 32, :],
                            )
                            nc.sync.dma_start(
                                ost_v[:, hh * 32 : hh * 32 + 32, 1, :],
                                o_all[64:128, hh * 32 : hh * 32 + 32, :],
                            )

            # ---- Phase 3: per-irrep fp16 linear, +sc, out ----
            ycat = big.tile([64, APC, 9], f32)

            with tc.tile_pool(name="lps", bufs=1, space="PSUM") as lps:
                lp0 = lps.tile([128, 512], f32, tag="lp0")
                lp1 = lps.tile([128, 512], f32, tag="lp1")
                lp2a = lps.tile([128, 512], f32, tag="lp2a")
                lp2b = lps.tile([128, 512], f32, tag="lp2b")
                nc.tensor.matmul(
                    lp0[0:64, 0:APC], wlin_sb[:, 0:64], ostage[:, :, 0:1],
                    start=True, stop=True,
                )
                nc.tensor.matmul(
                    lp1[0:64, 0 : 3 * APC], wlin_sb[:, 64:128],
                    ostage[:, :, 1:4], start=True, stop=True,
                )
                nc.tensor.matmul(
                    lp2a[0:64, 0:320], wlin_sb[:, 128:192],
                    ostage[:, 0:64, 4:9], start=True, stop=True,
                )
                nc.tensor.matmul(
                    lp2b[0:64, 0:320], wlin_sb[:, 128:192],
                    ostage[:, 64:APC, 4:9], start=True, stop=True,
                )
                # ycat = linear + sc  (fused adds straight from PSUM)
                nc.vector.tensor_add(
                    ycat[:, :, 0], lp0[0:64, 0:APC], sc_sb[:, :, 0]
                )
                nc.vector.tensor_add(
                    ycat[:, :, 1:4],
                    lp1[0:64, 0 : 3 * APC].rearrange("e (n d) -> e n d", d=3),
                    sc_sb[:, :, 1:4],
                )
                nc.vector.tensor_add(
                    ycat[:, 0:64, 4:9],
                    lp2a[0:64, 0:320].rearrange("e (n d) -> e n d", d=5),
                    sc_sb[:, 0:64, 4:9],
                )
                nc.vector.tensor_add(
                    ycat[:, 64:APC, 4:9],
                    lp2b[0:64, 0:320].rearrange("e (n d) -> e n d", d=5),
                    sc_sb[:, 64:APC, 4:9],
                )
            nc.sync.dma_start(yout_d[:], ycat[:])

    nc.finalize()
    return nc


def _prep_host(node_feats, sc, species, params):
    """Host-side constant folding + per-core layout prep."""
    node_feats = np.ascontiguousarray(np.asarray(node_feats, dtype=np.float32))
    sc = np.ascontiguousarray(np.asarray(sc, dtype=np.float32))
    species = np.asarray(species).astype(np.int64)

    uall = np.zeros((91, 360), dtype=np.float32)
    wg = np.zeros((N_ATOMS, C, 22), dtype=np.float32)
    wlin = np.zeros((64, 192), dtype=np.float32)
    for r in range(3):
        U3, U2, U1, W3, W2, W1, Wl = [np.asarray(t, np.float32) for t in params[r]]
        D, off = DIMS[r], DOFF[r]
        uall[0:81, off * 36 : (off + D) * 36] = U3.transpose(1, 2, 0, 3, 4).reshape(
            81, D * 36
        )
        u2block = np.zeros((81, D, 4), dtype=np.float32)
        u2block[:, :, 0:2] = U2.transpose(1, 2, 0, 3).reshape(81, D, 2)
        uall[0:81, 324 + off * 4 : 324 + (off + D) * 4] = u2block.reshape(81, D * 4)
        u1block = np.zeros((9, D, 4), dtype=np.float32)
        u1block[:, :, 2] = U1[:, :, 0].transpose(1, 0)
        uall[82:91, 324 + off * 4 : 324 + (off + D) * 4] = u1block.reshape(9, D * 4)
        wg[:, :, r * 7 + 0 : r * 7 + 4] = W3[species].transpose(0, 2, 1)
        wg[:, :, r * 7 + 4 : r * 7 + 6] = W2[species].transpose(0, 2, 1)
        wg[:, :, r * 7 + 6] = W1[species][:, 0, :]
        wlin[:, r * 64 : (r + 1) * 64] = Wl

    in_maps = []
    for core in range(N_CORES):
        a0 = core * APC
        nf = node_feats[a0 : a0 + APC].reshape(NB, 2, C, 9)
        wgc = wg[a0 : a0 + APC].reshape(NB, 2, C, 22)
        in_maps.append(
            {
                "xarr": np.ascontiguousarray(
                    nf.transpose(1, 2, 0, 3).reshape(128, NB, 9)
                ),
                "wgarr": np.ascontiguousarray(
                    wgc.transpose(1, 2, 0, 3).reshape(128, NB, 22)
                ),
                "scarr": np.ascontiguousarray(
                    sc[a0 : a0 + APC].transpose(1, 0, 2)
                ),
                "uall": uall.astype(np.float16),
                "wlin": wlin.astype(np.float16),
            }
        )
    return in_maps


def kernel(node_feats, sc, species, params, _trace=False):
    from concourse.bass_utils import run_bass_kernel_spmd

    if "nc" not in _compiled:
        _compiled["nc"] = _build_program()
    nc = _compiled["nc"]

    in_maps = _prep_host(node_feats, sc, species, params)
    res = run_bass_kernel_spmd(
        nc, in_maps, core_ids=list(range(N_CORES)), trace=_trace
    )
    _compiled["last_result"] = res

    out = np.empty((N_ATOMS, C, 9), dtype=np.float32)
    for core in range(N_CORES):
        a0 = core * APC
        out[a0 : a0 + APC] = res.results[core]["yout"].transpose(1, 0, 2)
    return out


# revision 30
# speedup vs baseline: 1.0201x; 1.0201x over previous
"""Trainium2 Bass kernel for an EquivariantProductBasisBlock (MACE-style
symmetric contraction + per-irrep linear + residual).

Problem shapes (hardcoded):
  N=1024 atoms, C=64 channels, K=9 total irrep dim, irreps dims (1,3,5),
  paths P3=4/P2=2/P1=1, E=10 species, 8 NeuronCores, data-parallel over N.

Decomposition (validated exactly against the reference):
  per sample s=(n,c), x = node_feats[n,c,:] (9-vec), per-irrep r species
  weights w3(4)/w2(2)/w1(1) gathered host-side into Wg[n,c,21]:
    T3[d] = sum_{a,b,i,p} U3[d,a,b,i,p] x_a x_b x_i w3_p
    T2[d] = sum_{a,i,p}   U2[d,a,i,p]   x_a x_i w2_p
    T1[d] = sum_{a}       U1[d,a,0]     x_a w1
  On device:  V[s] = [y2 = x (x) x (81) | pad | x (9)]  (sample-major fp16,
  PE-transposed per 128-sample chunk), then
    H[s, 0:360] = V @ Uall   (fp16 matmul, fp32 PSUM, ACT-copied to SBUF)
    step2 on DVE (fp16 2x mode): products [36 T3 | 4 T2+T1] per output d,
    three pairwise-add levels + one 5-wide reduce -> o_all
  Then per-irrep linear via 4 fp16 matmuls (contraction over channels) with
  fused +sc adds from PSUM; restage DMAs overlap phase 2 per atom-half.
"""

import numpy as np

N_ATOMS = 1024
C = 64
K = 9
DIMS = (1, 3, 5)
DOFF = (0, 1, 4)
P3, P2, P1 = 4, 2, 1
N_CORES = 8
APC = N_ATOMS // N_CORES  # atoms per core = 128
NB = APC // 2  # chunks per core = 64 (each chunk = 2 atoms x 64 channels)
GROUP = 4  # chunks per PSUM group
NGRP = NB // GROUP
SG = 16  # chunks per step2 supergroup
NSG = NB // SG

_compiled = {}


def _build_program():
    import concourse.bass as bass  # noqa: F401
    import concourse.tile as tile
    from concourse import bacc
    from concourse import mybir
    from concourse.masks import make_identity

    f32 = mybir.dt.float32
    f16 = mybir.dt.float16
    AX = mybir.AxisListType.X

    nc = bacc.Bacc(None)
    xarr_d = nc.dram_tensor("xarr", [128, NB, 9], f32, kind="ExternalInput")
    wg_d = nc.dram_tensor("wgarr", [128, NB, 22], f32, kind="ExternalInput")
    sc_d = nc.dram_tensor("scarr", [64, APC, 9], f32, kind="ExternalInput")
    uall_d = nc.dram_tensor("uall", [91, 360], f16, kind="ExternalInput")
    wlin_d = nc.dram_tensor("wlin", [64, 192], f16, kind="ExternalInput")
    yout_d = nc.dram_tensor("yout", [64, APC, 9], f32, kind="ExternalOutput")

    with tile.TileContext(nc) as tc:
        with (
            tc.tile_pool(name="const", bufs=1) as const,
            tc.tile_pool(name="big", bufs=1) as big,
            tc.tile_pool(name="vt", bufs=8) as vtp,
            tc.tile_pool(name="st2", bufs=2) as st2,
        ):
            ident16 = const.tile([128, 128], f16)
            make_identity(nc, ident16)
            ident32 = const.tile([128, 128], f32)
            make_identity(nc, ident32)
            u_sb = const.tile([91, 360], f16)
            nc.sync.dma_start(u_sb[:], uall_d[:])
            wlin_sb = const.tile([64, 192], f16)
            nc.sync.dma_start(wlin_sb[:], wlin_d[:])
            x_sb = const.tile([128, NB, 9], f32)
            nc.sync.dma_start(x_sb[:], xarr_d[:])
            wg_sb = const.tile([128, NB, 22], f32)
            nc.sync.dma_start(wg_sb[:], wg_d[:])
            sc_sb = const.tile([64, APC, 9], f32)
            nc.sync.dma_start(sc_sb[:], sc_d[:])

            vbig = big.tile([128, NB, 92], f16)
            ostage = big.tile([64, APC, 9], f16)
            ost_v = ostage.rearrange("p (n j) d -> p n j d", j=2)
            rallx = big.tile([128, NB, 108], f16)
            o_all = big.tile([128, NB, 9], f16)
            hsb = big.tile([128, NB, 360], f16)

            # ---- Phase 1: feature build (sample-major) ----
            # x rows of V at 4B-aligned offset 82; col 81 zeroed (junk
            # would hit the zero row of Uall otherwise; 0*NaN = NaN)
            nc.gpsimd.memset(vbig[:, :, 81:82], 0.0)
            nc.vector.tensor_copy(vbig[:, :, 82:91], x_sb)
            # y2[(a,b)] = x_a * x_b  (fp16 out for the PE); split so the
            # PE transposes can start after the first half
            for h in range(4):
                hk = NB // 4
                nc.vector.tensor_mul(
                    vbig[:, h * hk : (h + 1) * hk, 0:81].rearrange(
                        "p k (a b) -> p k a b", b=9
                    ),
                    x_sb[:, h * hk : (h + 1) * hk, :, None].broadcast_to(
                        (128, hk, 9, 9)
                    ),
                    x_sb[:, h * hk : (h + 1) * hk, None, :].broadcast_to(
                        (128, hk, 9, 9)
                    ),
                )

            # ---- Phase 2: pipelined PE transpose + matmul; h to SBUF ----
            with (
                tc.tile_pool(name="hps", bufs=3, space="PSUM") as hps,
                tc.tile_pool(name="vtps", bufs=2, space="PSUM") as vtps,
            ):
                # Warm-up PE ops absorb one-time GPSIMD (ident) / DMA (u_sb)
                # deps and start HAM warmup.
                warm = vtps.tile([92, 4, 128], f16, tag="vt_ps")
                nc.tensor.transpose(warm[:, 0, :], ident16[:, 0:92], ident16)
                warm2 = vtps.tile([128, 91], f16, tag="vt_ps")
                nc.tensor.transpose(warm2, u_sb[:, 0:128], ident16[0:91, 0:91])

                def emit_transposes(g):
                    vt_ps = vtps.tile([92, 4, 128], f16, tag="vt_ps")
                    for j in range(4):
                        nc.tensor.transpose(
                            vt_ps[:, j, :], vbig[:, g * 4 + j, :], ident16
                        )
                    vt_sb = vtp.tile([92, 4, 128], f16)
                    nc.scalar.copy(vt_sb, vt_ps)
                    return vt_sb

                def emit_step2(sg):
                    # t3t[dg] holds [36 T3 | 2 T2 | 1 T1] products; pairwise
                    # pre-add then one 20-wide DVE reduce per irrep writes
                    # o_all directly.
                    ks = sg * SG
                    t3t = st2.tile([128, SG, 9, 40], f16)
                    # rallx[(r,i,p)] = x_i * w3^r_p for this supergroup
                    for r in range(3):
                        nc.vector.tensor_mul(
                            rallx[
                                :, ks : ks + SG, r * 36 : (r + 1) * 36
                            ].rearrange("p k (i q) -> p k i q", q=4),
                            x_sb[:, ks : ks + SG, :, None].broadcast_to(
                                (128, SG, 9, 4)
                            ),
                            wg_sb[:, ks : ks + SG, None, r * 7 : r * 7 + 4]
                            .broadcast_to((128, SG, 9, 4)),
                        )
                    for r in range(3):
                        D, off = DIMS[r], DOFF[r]
                        h3 = hsb[
                            :, ks : ks + SG, off * 36 : (off + D) * 36
                        ].rearrange("p k (d f) -> p k d f", f=36)
                        m3 = rallx[:, ks : ks + SG, r * 36 : (r + 1) * 36][
                            :, :, None, :
                        ].broadcast_to((128, SG, D, 36))
                        nc.vector.tensor_mul(
                            t3t[:, :, off : off + D, 0:36], h3, m3
                        )
                        h21 = hsb[
                            :, ks : ks + SG, 324 + off * 4 : 324 + (off + D) * 4
                        ].rearrange("p k (d f) -> p k d f", f=4)
                        m21 = wg_sb[:, ks : ks + SG, r * 7 + 4 : r * 7 + 8][
                            :, :, None, :
                        ].broadcast_to((128, SG, D, 4))
                        nc.vector.tensor_mul(
                            t3t[:, :, off : off + D, 36:40], h21, m21
                        )
                    nc.vector.tensor_add(
                        t3t[:, :, :, 0:20],
                        t3t[:, :, :, 0:20],
                        t3t[:, :, :, 20:40],
                    )
                    nc.vector.tensor_add(
                        t3t[:, :, :, 0:10],
                        t3t[:, :, :, 0:10],
                        t3t[:, :, :, 10:20],
                    )
                    nc.vector.tensor_add(
                        t3t[:, :, :, 0:5],
                        t3t[:, :, :, 0:5],
                        t3t[:, :, :, 5:10],
                    )
                    with nc.allow_low_precision(
                        reason="5-term fp16 segment sum, O(1) values"
                    ):
                        nc.vector.reduce_sum(
                            out=o_all[:, ks : ks + SG, :],
                            in_=t3t[:, :, :, 0:5],
                            axis=AX,
                        )

                vt_cur = emit_transposes(0)
                for g in range(NGRP):
                    vt_nxt = emit_transposes(g + 1) if g + 1 < NGRP else None
                    for half in range(2):
                        ks = g * 4 + half * 2
                        h_t = hps.tile([128, 2, 512], f32)
                        for j in range(2):
                            nc.tensor.matmul(
                                h_t[:, j, 0:360],
                                vt_cur[0:91, half * 2 + j, :],
                                u_sb,
                                start=True,
                                stop=True,
                            )
                        nc.scalar.copy(
                            hsb[:, ks : ks + 2, 0:360], h_t[:, :, 0:360]
                        )
                    vt_cur = vt_nxt
                    if g % 4 == 3:
                        emit_step2(g // 4)
                        if g // 4 in (1, 3):
                            hh = (g // 4) // 2  # atom half 0 or 1
                            nc.sync.dma_start(
                                ost_v[:, hh * 32 : hh * 32 + 32, 0, :],
                                o_all[0:64, hh * 32 : hh * 32 + 32, :],
                            )
                            nc.sync.dma_start(
                                ost_v[:, hh * 32 : hh * 32 + 32, 1, :],
                                o_all[64:128, hh * 32 : hh * 32 + 32, :],
                            )

            # ---- Phase 3: per-irrep fp16 linear, +sc, out ----
            ycat = big.tile([64, APC, 9], f32)

            with tc.tile_pool(name="lps", bufs=1, space="PSUM") as lps:
                lp0 = lps.tile([128, 512], f32, tag="lp0")
                lp1 = lps.tile([128, 512], f32, tag="lp1")
                lp2a = lps.tile([128, 512], f32, tag="lp2a")
                lp2b = lps.tile([128, 512], f32, tag="lp2b")
                nc.tensor.matmul(
                    lp0[0:64, 0:APC], wlin_sb[:, 0:64], ostage[:, :, 0:1],
                    start=True, stop=True,
                )
                nc.tensor.matmul(
                    lp1[0:64, 0 : 3 * APC], wlin_sb[:, 64:128],
                    ostage[:, :, 1:4], start=True, stop=True,
                )
                nc.tensor.matmul(
                    lp2a[0:64, 0:320], wlin_sb[:, 128:192],
                    ostage[:, 0:64, 4:9], start=True, stop=True,
                )
                nc.tensor.matmul(
                    lp2b[0:64, 0:320], wlin_sb[:, 128:192],
                    ostage[:, 64:APC, 4:9], start=True, stop=True,
                )
                # ycat = linear + sc  (fused adds straight from PSUM)
                nc.vector.tensor_add(
                    ycat[:, :, 0], lp0[0:64, 0:APC], sc_sb[:, :, 0]
                )
                nc.vector.tensor_add(
                    ycat[:, :, 1:4],
                    lp1[0:64, 0 : 3 * APC].rearrange("e (n d) -> e n d", d=3),
                    sc_sb[:, :, 1:4],
                )
                nc.vector.tensor_add(
                    ycat[:, 0:64, 4:9],
                    lp2a[0:64, 0:320].rearrange("e (n d) -> e n d", d=5),
                    sc_sb[:, 0:64, 4:9],
                )
                nc.vector.tensor_add(
                    ycat[:, 64:APC, 4:9],
                    lp2b[0:64, 0:320].rearrange("e (n d) -> e n d", d=5),
                    sc_sb[:, 64:APC, 4:9],
                )
            nc.sync.dma_start(yout_d[:], ycat[:])

    nc.finalize()
    return nc


def _prep_host(node_feats, sc, species, params):
    """Host-side constant folding + per-core layout prep."""
    node_feats = np.ascontiguousarray(np.asarray(node_feats, dtype=np.float32))
    sc = np.ascontiguousarray(np.asarray(sc, dtype=np.float32))
    species = np.asarray(species).astype(np.int64)

    uall = np.zeros((91, 360), dtype=np.float32)
    wg = np.zeros((N_ATOMS, C, 22), dtype=np.float32)
    wlin = np.zeros((64, 192), dtype=np.float32)
    for r in range(3):
        U3, U2, U1, W3, W2, W1, Wl = [np.asarray(t, np.float32) for t in params[r]]
        D, off = DIMS[r], DOFF[r]
        uall[0:81, off * 36 : (off + D) * 36] = U3.transpose(1, 2, 0, 3, 4).reshape(
            81, D * 36
        )
        u2block = np.zeros((81, D, 4), dtype=np.float32)
        u2block[:, :, 0:2] = U2.transpose(1, 2, 0, 3).reshape(81, D, 2)
        uall[0:81, 324 + off * 4 : 324 + (off + D) * 4] = u2block.reshape(81, D * 4)
        u1block = np.zeros((9, D, 4), dtype=np.float32)
        u1block[:, :, 2] = U1[:, :, 0].transpose(1, 0)
        uall[82:91, 324 + off * 4 : 324 + (off + D) * 4] = u1block.reshape(9, D * 4)
        wg[:, :, r * 7 + 0 : r * 7 + 4] = W3[species].transpose(0, 2, 1)
        wg[:, :, r * 7 + 4 : r * 7 + 6] = W2[species].transpose(0, 2, 1)
        wg[:, :, r * 7 + 6] = W1[species][:, 0, :]
        wlin[:, r * 64 : (r + 1) * 64] = Wl

    in_maps = []
    for core in range(N_CORES):
        a0 = core * APC
        nf = node_feats[a0 : a0 + APC].reshape(NB, 2, C, 9)
        wgc = wg[a0 : a0 + APC].reshape(NB, 2, C, 22)
        in_maps.append(
            {
                "xarr": np.ascontiguousarray(
                    nf.transpose(1, 2, 0, 3).reshape(128, NB, 9)
                ),
                "wgarr": np.ascontiguousarray(
                    wgc.transpose(1, 2, 0, 3).reshape(128, NB, 22)
                ),
                "scarr": np.ascontiguousarray(
                    sc[a0 : a0 + APC].transpose(1, 0, 2)
                ),
                "uall": uall.astype(np.float16),
                "wlin": wlin.astype(np.float16),
            }
        )
    return in_maps


def kernel(node_feats, sc, species, params, _trace=False):
    from concourse.bass_utils import run_bass_kernel_spmd

    if "nc" not in _compiled:
        _compiled["nc"] = _build_program()
    nc = _compiled["nc"]

    in_maps = _prep_host(node_feats, sc, species, params)
    res = run_bass_kernel_spmd(
        nc, in_maps, core_ids=list(range(N_CORES)), trace=_trace
    )
    _compiled["last_result"] = res

    out = np.empty((N_ATOMS, C, 9), dtype=np.float32)
    for core in range(N_CORES):
        a0 = core * APC
        out[a0 : a0 + APC] = res.results[core]["yout"].transpose(1, 0, 2)
    return out


# revision 31
# speedup vs baseline: 1.0284x; 1.0082x over previous
"""Trainium2 Bass kernel for an EquivariantProductBasisBlock (MACE-style
symmetric contraction + per-irrep linear + residual).

Problem shapes (hardcoded):
  N=1024 atoms, C=64 channels, K=9 total irrep dim, irreps dims (1,3,5),
  paths P3=4/P2=2/P1=1, E=10 species, 8 NeuronCores, data-parallel over N.

Decomposition (validated exactly against the reference):
  per sample s=(n,c), x = node_feats[n,c,:] (9-vec), per-irrep r species
  weights w3(4)/w2(2)/w1(1) gathered host-side into Wg[n,c,21]:
    T3[d] = sum_{a,b,i,p} U3[d,a,b,i,p] x_a x_b x_i w3_p
    T2[d] = sum_{a,i,p}   U2[d,a,i,p]   x_a x_i w2_p
    T1[d] = sum_{a}       U1[d,a,0]     x_a w1
  On device:  V[s] = [y2 = x (x) x (81) | pad | x (9)]  (sample-major fp16,
  PE-transposed per 128-sample chunk), then
    H[s, 0:360] = V @ Uall   (fp16 matmul, fp32 PSUM, ACT-copied to SBUF)
    step2 on DVE (fp16 2x mode): products [36 T3 | 4 T2+T1] per output d,
    three pairwise-add levels + one 5-wide reduce -> o_all
  Then per-irrep linear via 4 fp16 matmuls (contraction over channels) with
  fused +sc adds from PSUM; restage DMAs overlap phase 2 per atom-half.
"""

import numpy as np

N_ATOMS = 1024
C = 64
K = 9
DIMS = (1, 3, 5)
DOFF = (0, 1, 4)
P3, P2, P1 = 4, 2, 1
N_CORES = 8
APC = N_ATOMS // N_CORES  # atoms per core = 128
NB = APC // 2  # chunks per core = 64 (each chunk = 2 atoms x 64 channels)
GROUP = 4  # chunks per PSUM group
NGRP = NB // GROUP
SG = 16  # chunks per step2 supergroup
NSG = NB // SG

_compiled = {}


def _build_program():
    import concourse.bass as bass  # noqa: F401
    import concourse.tile as tile
    from concourse import bacc
    from concourse import mybir
    from concourse.masks import make_identity

    f32 = mybir.dt.float32
    f16 = mybir.dt.float16
    AX = mybir.AxisListType.X

    nc = bacc.Bacc(None)
    xarr_d = nc.dram_tensor("xarr", [128, NB, 9], f32, kind="ExternalInput")
    wg_d = nc.dram_tensor("wgarr", [128, NB, 22], f32, kind="ExternalInput")
    sc_d = nc.dram_tensor("scarr", [64, APC, 9], f32, kind="ExternalInput")
    uall_d = nc.dram_tensor("uall", [91, 360], f16, kind="ExternalInput")
    wlin_d = nc.dram_tensor("wlin", [64, 192], f16, kind="ExternalInput")
    yout_d = nc.dram_tensor("yout", [64, APC, 9], f32, kind="ExternalOutput")

    with tile.TileContext(nc) as tc:
        with (
            tc.tile_pool(name="const", bufs=1) as const,
            tc.tile_pool(name="big", bufs=1) as big,
            tc.tile_pool(name="vt", bufs=8) as vtp,
            tc.tile_pool(name="st2", bufs=3) as st2,
        ):
            ident16 = const.tile([128, 128], f16)
            make_identity(nc, ident16)
            ident32 = const.tile([128, 128], f32)
            make_identity(nc, ident32)
            u_sb = const.tile([91, 360], f16)
            nc.sync.dma_start(u_sb[:], uall_d[:])
            wlin_sb = const.tile([64, 192], f16)
            nc.sync.dma_start(wlin_sb[:], wlin_d[:])
            x_sb = const.tile([128, NB, 9], f32)
            nc.sync.dma_start(x_sb[:], xarr_d[:])
            wg_sb = const.tile([128, NB, 22], f32)
            nc.sync.dma_start(wg_sb[:], wg_d[:])
            sc_sb = const.tile([64, APC, 9], f32)
            nc.sync.dma_start(sc_sb[:], sc_d[:])

            vbig = big.tile([128, NB, 92], f16)
            ostage = big.tile([64, APC, 9], f16)
            ost_v = ostage.rearrange("p (n j) d -> p n j d", j=2)
            rallx = big.tile([128, NB, 108], f16)
            o_all = big.tile([128, NB, 9], f16)
            hsb = big.tile([128, NB, 360], f16)

            # ---- Phase 1: feature build (sample-major) ----
            # x rows of V at 4B-aligned offset 82; col 81 zeroed (junk
            # would hit the zero row of Uall otherwise; 0*NaN = NaN)
            nc.gpsimd.memset(vbig[:, :, 81:82], 0.0)
            nc.vector.tensor_copy(vbig[:, :, 82:91], x_sb)
            # y2[(a,b)] = x_a * x_b  (fp16 out for the PE); split so the
            # PE transposes can start after the first half
            for h in range(4):
                hk = NB // 4
                nc.vector.tensor_mul(
                    vbig[:, h * hk : (h + 1) * hk, 0:81].rearrange(
                        "p k (a b) -> p k a b", b=9
                    ),
                    x_sb[:, h * hk : (h + 1) * hk, :, None].broadcast_to(
                        (128, hk, 9, 9)
                    ),
                    x_sb[:, h * hk : (h + 1) * hk, None, :].broadcast_to(
                        (128, hk, 9, 9)
                    ),
                )

            # ---- Phase 2: pipelined PE transpose + matmul; h to SBUF ----
            with (
                tc.tile_pool(name="hps", bufs=3, space="PSUM") as hps,
                tc.tile_pool(name="vtps", bufs=2, space="PSUM") as vtps,
            ):
                # Warm-up PE ops absorb one-time GPSIMD (ident) / DMA (u_sb)
                # deps and start HAM warmup.
                warm = vtps.tile([92, 4, 128], f16, tag="vt_ps")
                nc.tensor.transpose(warm[:, 0, :], ident16[:, 0:92], ident16)
                warm2 = vtps.tile([128, 91], f16, tag="vt_ps")
                nc.tensor.transpose(warm2, u_sb[:, 0:128], ident16[0:91, 0:91])

                def emit_transposes(g):
                    vt_ps = vtps.tile([92, 4, 128], f16, tag="vt_ps")
                    for j in range(4):
                        nc.tensor.transpose(
                            vt_ps[:, j, :], vbig[:, g * 4 + j, :], ident16
                        )
                    vt_sb = vtp.tile([92, 4, 128], f16)
                    nc.scalar.copy(vt_sb, vt_ps)
                    return vt_sb

                def emit_step2(sg):
                    # t3t[dg] holds [36 T3 | 2 T2 | 1 T1] products; pairwise
                    # pre-add then one 20-wide DVE reduce per irrep writes
                    # o_all directly.
                    ks = sg * SG
                    t3t = st2.tile([128, SG, 9, 40], f16)
                    # rallx[(r,i,p)] = x_i * w3^r_p for this supergroup
                    for r in range(3):
                        nc.vector.tensor_mul(
                            rallx[
                                :, ks : ks + SG, r * 36 : (r + 1) * 36
                            ].rearrange("p k (i q) -> p k i q", q=4),
                            x_sb[:, ks : ks + SG, :, None].broadcast_to(
                                (128, SG, 9, 4)
                            ),
                            wg_sb[:, ks : ks + SG, None, r * 7 : r * 7 + 4]
                            .broadcast_to((128, SG, 9, 4)),
                        )
                    for r in range(3):
                        D, off = DIMS[r], DOFF[r]
                        h3 = hsb[
                            :, ks : ks + SG, off * 36 : (off + D) * 36
                        ].rearrange("p k (d f) -> p k d f", f=36)
                        m3 = rallx[:, ks : ks + SG, r * 36 : (r + 1) * 36][
                            :, :, None, :
                        ].broadcast_to((128, SG, D, 36))
                        nc.vector.tensor_mul(
                            t3t[:, :, off : off + D, 0:36], h3, m3
                        )
                        h21 = hsb[
                            :, ks : ks + SG, 324 + off * 4 : 324 + (off + D) * 4
                        ].rearrange("p k (d f) -> p k d f", f=4)
                        m21 = wg_sb[:, ks : ks + SG, r * 7 + 4 : r * 7 + 8][
                            :, :, None, :
                        ].broadcast_to((128, SG, D, 4))
                        nc.vector.tensor_mul(
                            t3t[:, :, off : off + D, 36:40], h21, m21
                        )
                    nc.vector.tensor_add(
                        t3t[:, :, :, 0:20],
                        t3t[:, :, :, 0:20],
                        t3t[:, :, :, 20:40],
                    )
                    nc.vector.tensor_add(
                        t3t[:, :, :, 0:10],
                        t3t[:, :, :, 0:10],
                        t3t[:, :, :, 10:20],
                    )
                    nc.vector.tensor_add(
                        t3t[:, :, :, 0:5],
                        t3t[:, :, :, 0:5],
                        t3t[:, :, :, 5:10],
                    )
                    with nc.allow_low_precision(
                        reason="5-term fp16 segment sum, O(1) values"
                    ):
                        nc.vector.reduce_sum(
                            out=o_all[:, ks : ks + SG, :],
                            in_=t3t[:, :, :, 0:5],
                            axis=AX,
                        )

                vt_cur = emit_transposes(0)
                for g in range(NGRP):
                    vt_nxt = emit_transposes(g + 1) if g + 1 < NGRP else None
                    for half in range(2):
                        ks = g * 4 + half * 2
                        h_t = hps.tile([128, 2, 512], f32)
                        for j in range(2):
                            nc.tensor.matmul(
                                h_t[:, j, 0:360],
                                vt_cur[0:91, half * 2 + j, :],
                                u_sb,
                                start=True,
                                stop=True,
                            )
                        nc.scalar.copy(
                            hsb[:, ks : ks + 2, 0:360], h_t[:, :, 0:360]
                        )
                    vt_cur = vt_nxt
                    if g % 4 == 3:
                        emit_step2(g // 4)
                        if g // 4 in (1, 3):
                            hh = (g // 4) // 2  # atom half 0 or 1
                            nc.sync.dma_start(
                                ost_v[:, hh * 32 : hh * 32 + 32, 0, :],
                                o_all[0:64, hh * 32 : hh * 32 + 32, :],
                            )
                            nc.sync.dma_start(
                                ost_v[:, hh * 32 : hh * 32 + 32, 1, :],
                                o_all[64:128, hh * 32 : hh * 32 + 32, :],
                            )

            # ---- Phase 3: per-irrep fp16 linear, +sc, out ----
            ycat = big.tile([64, APC, 9], f32)

            with tc.tile_pool(name="lps", bufs=1, space="PSUM") as lps:
                lp0 = lps.tile([128, 512], f32, tag="lp0")
                lp1 = lps.tile([128, 512], f32, tag="lp1")
                lp2a = lps.tile([128, 512], f32, tag="lp2a")
                lp2b = lps.tile([128, 512], f32, tag="lp2b")
                nc.tensor.matmul(
                    lp0[0:64, 0:APC], wlin_sb[:, 0:64], ostage[:, :, 0:1],
                    start=True, stop=True,
                )
                nc.tensor.matmul(
                    lp1[0:64, 0 : 3 * APC], wlin_sb[:, 64:128],
                    ostage[:, :, 1:4], start=True, stop=True,
                )
                nc.tensor.matmul(
                    lp2a[0:64, 0:320], wlin_sb[:, 128:192],
                    ostage[:, 0:64, 4:9], start=True, stop=True,
                )
                nc.tensor.matmul(
                    lp2b[0:64, 0:320], wlin_sb[:, 128:192],
                    ostage[:, 64:APC, 4:9], start=True, stop=True,
                )
                # ycat = linear + sc  (fused adds straight from PSUM)
                nc.vector.tensor_add(
                    ycat[:, :, 0], lp0[0:64, 0:APC], sc_sb[:, :, 0]
                )
                nc.vector.tensor_add(
                    ycat[:, :, 1:4],
                    lp1[0:64, 0 : 3 * APC].rearrange("e (n d) -> e n d", d=3),
                    sc_sb[:, :, 1:4],
                )
                nc.vector.tensor_add(
                    ycat[:, 0:64, 4:9],
                    lp2a[0:64, 0:320].rearrange("e (n d) -> e n d", d=5),
                    sc_sb[:, 0:64, 4:9],
                )
                nc.vector.tensor_add(
                    ycat[:, 64:APC, 4:9],
                    lp2b[0:64, 0:320].rearrange("e (n d) -> e n d", d=5),
                    sc_sb[:, 64:APC, 4:9],
                )
            nc.sync.dma_start(yout_d[:], ycat[:])

    nc.finalize()
    return nc


def _prep_host(node_feats, sc, species, params):
    """Host-side constant folding + per-core layout prep."""
    node_feats = np.ascontiguousarray(np.asarray(node_feats, dtype=np.float32))
    sc = np.ascontiguousarray(np.asarray(sc, dtype=np.float32))
    species = np.asarray(species).astype(np.int64)

    uall = np.zeros((91, 360), dtype=np.float32)
    wg = np.zeros((N_ATOMS, C, 22), dtype=np.float32)
    wlin = np.zeros((64, 192), dtype=np.float32)
    for r in range(3):
        U3, U2, U1, W3, W2, W1, Wl = [np.asarray(t, np.float32) for t in params[r]]
        D, off = DIMS[r], DOFF[r]
        uall[0:81, off * 36 : (off + D) * 36] = U3.transpose(1, 2, 0, 3, 4).reshape(
            81, D * 36
        )
        u2block = np.zeros((81, D, 4), dtype=np.float32)
        u2block[:, :, 0:2] = U2.transpose(1, 2, 0, 3).reshape(81, D, 2)
        uall[0:81, 324 + off * 4 : 324 + (off + D) * 4] = u2block.reshape(81, D * 4)
        u1block = np.zeros((9, D, 4), dtype=np.float32)
        u1block[:, :, 2] = U1[:, :, 0].transpose(1, 0)
        uall[82:91, 324 + off * 4 : 324 + (off + D) * 4] = u1block.reshape(9, D * 4)
        wg[:, :, r * 7 + 0 : r * 7 + 4] = W3[species].transpose(0, 2, 1)
        wg[:, :, r * 7 + 4 : r * 7 + 6] = W2[species].transpose(0, 2, 1)
        wg[:, :, r * 7 + 6] = W1[species][:, 0, :]
        wlin[:, r * 64 : (r + 1) * 64] = Wl

    in_maps = []
    for core in range(N_CORES):
        a0 = core * APC
        nf = node_feats[a0 : a0 + APC].reshape(NB, 2, C, 9)
        wgc = wg[a0 : a0 + APC].reshape(NB, 2, C, 22)
        in_maps.append(
            {
                "xarr": np.ascontiguousarray(
                    nf.transpose(1, 2, 0, 3).reshape(128, NB, 9)
                ),
                "wgarr": np.ascontiguousarray(
                    wgc.transpose(1, 2, 0, 3).reshape(128, NB, 22)
                ),
                "scarr": np.ascontiguousarray(
                    sc[a0 : a0 + APC].transpose(1, 0, 2)
                ),
                "uall": uall.astype(np.float16),
                "wlin": wlin.astype(np.float16),
            }
        )
    return in_maps


def kernel(node_feats, sc, species, params, _trace=False):
    from concourse.bass_utils import run_bass_kernel_spmd

    if "nc" not in _compiled:
        _compiled["nc"] = _build_program()
    nc = _compiled["nc"]

    in_maps = _prep_host(node_feats, sc, species, params)
    res = run_bass_kernel_spmd(
        nc, in_maps, core_ids=list(range(N_CORES)), trace=_trace
    )
    _compiled["last_result"] = res

    out = np.empty((N_ATOMS, C, 9), dtype=np.float32)
    for core in range(N_CORES):
        a0 = core * APC
        out[a0 : a0 + APC] = res.results[core]["yout"].transpose(1, 0, 2)
    return out
